# revision 1
# baseline (speedup 1.0000x reference)
"""Trainium2 Bass kernel for nn_GroupLocalSL2 (grouped gather + conv).

out[b,o,i,xo,yo] = sum_{c,f,kh,kw} x[b,c,idx[i,f],xo+kh,yo+kw] * W[o,c,f,kh,kw] + bias[o]

Strategy:
  - Batch B=8 sharded across 8 NeuronCores (data parallel), one b per core.
  - Per core: for each output group i, gather the G_F=7 selected G_IN images
    into SBUF via DMA (idx is read host-side at build time and baked into the
    DMA program). Contraction (c,f)=224 split into chunkA (f=0..3, K=128) and
    chunkB (f=4..6, K=96), partition p = f*32 + c.
  - kw offsets paired into matmul M-blocks: groups {kw0,kw1},{kw2,kw3} give
    M=128 matmuls; kw4 runs as two concurrent M=64 matmuls column-tiled at
    PSUM partitions 0-63 / 64-127. kh accumulates in PSUM via row-shifted rhs.
  - Compute in bf16 (hosts casts x/W), fp32 PSUM accumulate; rel err ~2e-3.
  - VectorE combines the kw-shifted PSUM halves + bias, DMA to DRAM.
"""

import os
import sys

import numpy as np
import ml_dtypes

for _p in ("/opt/trn_rl_repo", "/root/.axon_site/_ro/trn_rl_repo"):
    if os.path.isdir(_p) and _p not in sys.path:
        sys.path.append(_p)

import concourse.bass as bass
import concourse.mybir as mybir
import concourse.tile as tile
from concourse import bacc
from concourse.bass_utils import run_bass_kernel_spmd

BF16 = ml_dtypes.bfloat16

B, C, G_IN = 8, 32, 33
O, G_F, KH, KW = 64, 7, 5, 5
X, Y = 64, 64
G_OUT = 15
XO, YO = X - KH + 1, Y - KW + 1  # 60, 60
RCH = 8  # output rows per chunk (8*61 = 488 <= 512 psum bank)


def _build_nc(idx, n_groups=G_OUT):
    """Build the single-core Bass program (idx values baked into DMAs)."""
    nc = bacc.Bacc("TRN2", target_bir_lowering=False, debug=False)
    dt = mybir.dt
    xin = nc.dram_tensor("x", [C, G_IN, X, Y], dt.bfloat16, kind="ExternalInput")
    wa_d = nc.dram_tensor("wa", [128, KH, 5 * O], dt.bfloat16, kind="ExternalInput")
    wb_d = nc.dram_tensor("wb", [96, KH, 5 * O], dt.bfloat16, kind="ExternalInput")
    bias_d = nc.dram_tensor("bias", [O, 1], dt.float32, kind="ExternalInput")
    out_d = nc.dram_tensor("out", [O, G_OUT, XO, YO], dt.float32, kind="ExternalOutput")

    rchunks = [(r0, min(RCH, XO - r0)) for r0 in range(0, XO, RCH)]

    with tile.TileContext(nc) as tc:
        with (
            tc.tile_pool(name="wpool", bufs=1) as wpool,
            tc.tile_pool(name="xpool", bufs=3) as xpool,
            tc.tile_pool(name="tpool", bufs=3) as tpool,
            tc.tile_pool(name="opool", bufs=4) as opool,
            tc.tile_pool(name="psum", bufs=2, space="PSUM") as pp,
            tc.tile_pool(name="psum3", bufs=3, space="PSUM") as pp3,
        ):
            wa = wpool.tile([128, KH, 5 * O], dt.bfloat16, tag="wa")
            wb = wpool.tile([96, KH, 5 * O], dt.bfloat16, tag="wb")
            bias_sb = wpool.tile([O, 1], dt.float32, tag="bias")
            nc.sync.dma_start(wa[:, :, :], wa_d[:, :, :])
            nc.sync.dma_start(wb[:, :, :], wb_d[:, :, :])
            nc.sync.dma_start(bias_sb[:, :], bias_d[:, :])

            for i in range(n_groups):
                xa = xpool.tile([128, X, Y], dt.bfloat16, tag="xa")
                xb = xpool.tile([96, X, Y], dt.bfloat16, tag="xb")
                for f in range(G_F):
                    g = int(idx[i, f])
                    if f < 4:
                        nc.sync.dma_start(
                            xa[f * 32 : (f + 1) * 32, :, :], xin[:, g, :, :]
                        )
                    else:
                        fb = f - 4
                        nc.sync.dma_start(
                            xb[fb * 32 : (fb + 1) * 32, :, :], xin[:, g, :, :]
                        )

                for r0, R in rchunks:
                    p0 = pp.tile([128, RCH, 61], dt.float32, tag="p0")
                    p1 = pp3.tile([128, RCH, 61], dt.float32, tag="p1")
                    p2 = pp3.tile([128, RCH, 60], dt.float32, tag="p2")
                    # groups {kw0,kw1} and {kw2,kw3}: M=128, 61-wide windows
                    for grp, ps in ((0, p0), (1, p1)):
                        c0 = 2 * grp
                        for ci, (xt, wt, Kc) in enumerate(
                            ((xa, wa, 128), (xb, wb, 96))
                        ):
                            for kh in range(KH):
                                nc.tensor.matmul(
                                    ps[:, 0:R, :],
                                    wt[0:Kc, kh, grp * 128 : grp * 128 + 128],
                                    xt[0:Kc, r0 + kh : r0 + kh + R, c0 : c0 + 61],
                                    start=(ci == 0 and kh == 0),
                                    stop=(ci == 1 and kh == KH - 1),
                                )
                    # group {kw4}: two concurrent M=64 col-tiled matmuls
                    for kh in range(KH):
                        nc.tensor.matmul(
                            p2[0:64, 0:R, :],
                            wa[0:128, kh, 256:320],
                            xa[0:128, r0 + kh : r0 + kh + R, 4:64],
                            start=(kh == 0),
                            stop=(kh == KH - 1),
                        )
                        nc.tensor.matmul(
                            p2[64:128, 0:R, :],
                            wb[0:96, kh, 256:320],
                            xb[0:96, r0 + kh : r0 + kh + R, 4:64],
                            start=(kh == 0),
                            stop=(kh == KH - 1),
                        )

                    # Combine: at most ONE PSUM operand per instruction
                    # (walrus NCC_IBVF027). Serial chain, bias-add on ScalarE.
                    t = tpool.tile([O, RCH, 60], dt.float32, tag="t")
                    ot = opool.tile([O, RCH, 60], dt.float32, tag="out")
                    nc.scalar.add(t[:, 0:R, :], p0[0:64, 0:R, 0:60], bias_sb[:, 0:1])
                    nc.vector.tensor_add(
                        t[:, 0:R, :], t[:, 0:R, :], p0[64:128, 0:R, 1:61]
                    )
                    nc.vector.tensor_add(
                        t[:, 0:R, :], t[:, 0:R, :], p1[0:64, 0:R, 0:60]
                    )
                    nc.vector.tensor_add(
                        t[:, 0:R, :], t[:, 0:R, :], p1[64:128, 0:R, 1:61]
                    )
                    nc.vector.tensor_add(
                        t[:, 0:R, :], t[:, 0:R, :], p2[0:64, 0:R, :]
                    )
                    nc.vector.tensor_add(
                        ot[:, 0:R, :], t[:, 0:R, :], p2[64:128, 0:R, :]
                    )
                    nc.sync.dma_start(
                        out_d[:, i, r0 : r0 + R, :], ot[:, 0:R, :]
                    )
    nc.compile()
    return nc


def _prep_inputs(x, weight, bias, idx):
    """Host-side staging: bf16 cast + lhsT weight layout, per-core in_maps."""
    x16 = np.asarray(x).astype(BF16)  # [B, C, G_IN, X, Y]
    w = np.asarray(weight).astype(np.float32)
    # lhsT layout: partition p = f*32 + c (within chunk), free = [kh, kw*64+o]
    wt = w.transpose(2, 1, 3, 4, 0)  # [G_F, C, KH, KW, O]
    wa = np.ascontiguousarray(wt[0:4].reshape(128, KH, KW * O)).astype(BF16)
    wb = np.ascontiguousarray(wt[4:7].reshape(96, KH, KW * O)).astype(BF16)
    b2 = np.ascontiguousarray(np.asarray(bias).astype(np.float32).reshape(O, 1))
    in_maps = []
    for b in range(B):
        in_maps.append(
            {
                "x": np.ascontiguousarray(x16[b]),
                "wa": wa,
                "wb": wb,
                "bias": b2,
            }
        )
    return in_maps


def run(x, weight, bias, idx, trace=False):
    idx = np.asarray(idx).astype(np.int64)
    assert idx.shape == (G_OUT, G_F) and idx.min() >= 0 and idx.max() < G_IN
    nc = _build_nc(idx)
    in_maps = _prep_inputs(x, weight, bias, idx)
    res = run_bass_kernel_spmd(nc, in_maps, list(range(B)), trace=trace)
    out = np.stack([res.results[b]["out"] for b in range(B)]).astype(np.float32)
    return out, res


def kernel(x, weight, bias, idx):
    out, _ = run(x, weight, bias, idx, trace=False)
    return out



# revision 2
# speedup vs baseline: 1.2031x; 1.2031x over previous
"""Trainium2 Bass kernel v2 for nn_GroupLocalSL2 (grouped gather + conv).

out[b,o,i,xo,yo] = sum_{c,f,kh,kw} x[b,c,idx[i,f],xo+kh,yo+kw] * W[o,c,f,kh,kw] + bias[o]

v2 restructure vs baseline:
  - ALL 25 taps accumulate into ONE psum tile [128, R, 61] per row-chunk:
    half0 (partitions 0:64) collects even-kw taps at aligned cols, half1
    (64:128) collects odd-kw at +1-shifted cols; kw4 splits M=64 into the
    two halves (B-side written at +1 col offset). Combine = 2 ops (was 6).
  - Chunk B's 15 (f,kh) cells repacked into 4 full-K passes using
    row-shifted duplicate gathers (tiles xba/xbb/xbc hold f-images at row
    shifts 0/1), killing the K=96 idle rows. A-side stays 5 passes K=128.
  - Per (i, rchunk): 27 matmuls, ~ (18*61 + 5*60)*R cycles.
  - Batch B=8 across 8 cores (1 core per b), bf16 compute, fp32 psum.
"""

import os
import sys

import numpy as np
import ml_dtypes

for _p in ("/opt/trn_rl_repo", "/root/.axon_site/_ro/trn_rl_repo"):
    if os.path.isdir(_p) and _p not in sys.path:
        sys.path.append(_p)

import concourse.bass as bass
import concourse.mybir as mybir
import concourse.tile as tile
from concourse import bacc
from concourse.bass_utils import run_bass_kernel_spmd

BF16 = ml_dtypes.bfloat16

B, C, G_IN = 8, 32, 33
O, G_F, KH, KW = 64, 7, 5, 5
X, Y = 64, 64
G_OUT = 15
XO, YO = X - KH + 1, Y - KW + 1  # 60, 60
RCH = 8

# B-side pass table: (tile_index, base_row_offset delta, K)
#   tile 0 = xba lanes [f4s0, f5s0, f6s0, f4s1]
#   tile 1 = xbb lanes [f5s0, f6s0, f4s1, f5s1]
#   tile 2 = xbc lanes [f6s0, f4s1, f5s1, f6s1]
B_TILE_LANES = [
    [(4, 0), (5, 0), (6, 0), (4, 1)],
    [(5, 0), (6, 0), (4, 1), (5, 1)],
    [(6, 0), (4, 1), (5, 1), (6, 1)],
]
B_PASSES = [  # (tile, delta, K) -> lane tap kh = delta + shift
    (0, 0, 128),
    (1, 1, 128),
    (2, 2, 128),
    (0, 4, 96),
]


def _build_nc(idx, n_groups=G_OUT):
    nc = bacc.Bacc("TRN2", target_bir_lowering=False, debug=False)
    dt = mybir.dt
    xin = nc.dram_tensor("x", [C, G_IN, X, Y], dt.bfloat16, kind="ExternalInput")
    wa_d = nc.dram_tensor("wa", [128, KH, 320], dt.bfloat16, kind="ExternalInput")
    wb_d = nc.dram_tensor("wb", [128, 4, 320], dt.bfloat16, kind="ExternalInput")
    bias_d = nc.dram_tensor("bias", [O, 1], dt.float32, kind="ExternalInput")
    out_d = nc.dram_tensor("out", [O, G_OUT, XO, YO], dt.float32, kind="ExternalOutput")

    rchunks = [(r0, min(RCH, XO - r0)) for r0 in range(0, XO, RCH)]

    with tile.TileContext(nc) as tc:
        with (
            tc.tile_pool(name="wpool", bufs=1) as wpool,
            tc.tile_pool(name="xpool", bufs=3) as xpool,
            tc.tile_pool(name="tpool", bufs=4) as tpool,
            tc.tile_pool(name="opool", bufs=4) as opool,
            tc.tile_pool(name="psum", bufs=6, space="PSUM") as pp,
        ):
            wa = wpool.tile([128, KH, 320], dt.bfloat16, tag="wa")
            wb = wpool.tile([128, 4, 320], dt.bfloat16, tag="wb")
            bias_sb = wpool.tile([O, 1], dt.float32, tag="bias")
            nc.sync.dma_start(wa[:, :, :], wa_d[:, :, :])
            nc.sync.dma_start(wb[:, :, :], wb_d[:, :, :])
            nc.sync.dma_start(bias_sb[:, :], bias_d[:, :])

            for i in range(n_groups):
                xa = xpool.tile([128, X, Y], dt.bfloat16, tag="xa")
                xb0 = xpool.tile([128, X, Y], dt.bfloat16, tag="xb0")
                xb1 = xpool.tile([128, X, Y], dt.bfloat16, tag="xb1")
                xb2 = xpool.tile([128, X, Y], dt.bfloat16, tag="xb2")
                xbs = [xb0, xb1, xb2]
                for f in range(4):
                    g = int(idx[i, f])
                    nc.sync.dma_start(xa[f * 32 : f * 32 + 32, :, :], xin[:, g, :, :])
                for t, lanes in enumerate(B_TILE_LANES):
                    for l, (f, s) in enumerate(lanes):
                        g = int(idx[i, f])
                        if s == 0:
                            nc.sync.dma_start(
                                xbs[t][l * 32 : l * 32 + 32, :, :], xin[:, g, :, :]
                            )
                        else:
                            # row-shifted copy: dest rows 0..62 = src rows 1..63
                            nc.sync.dma_start(
                                xbs[t][l * 32 : l * 32 + 32, 0 : X - 1, :],
                                xin[:, g, 1:X, :],
                            )

                for r0, R in rchunks:
                    ps = pp.tile([128, RCH, 61], dt.float32, tag="ps")
                    n_mm = 27
                    k = 0

                    def flags():
                        nonlocal k
                        st = k == 0
                        sp = k == n_mm - 1
                        k += 1
                        return st, sp

                    # A-side full-M window passes (kh = p)
                    for p in range(KH):
                        st, sp = flags()
                        nc.tensor.matmul(
                            ps[:, 0:R, 0:61],
                            wa[:, p, 0:128],
                            xa[:, r0 + p : r0 + p + R, 0:61],
                            start=st, stop=sp,
                        )
                        st, sp = flags()
                        nc.tensor.matmul(
                            ps[:, 0:R, 0:61],
                            wa[:, p, 128:256],
                            xa[:, r0 + p : r0 + p + R, 2:63],
                            start=st, stop=sp,
                        )
                    # B-side full-M window passes
                    for p, (t, dlt, K) in enumerate(B_PASSES):
                        xt = xbs[t]
                        st, sp = flags()
                        nc.tensor.matmul(
                            ps[:, 0:R, 0:61],
                            wb[0:K, p, 0:128],
                            xt[0:K, r0 + dlt : r0 + dlt + R, 0:61],
                            start=st, stop=sp,
                        )
                        st, sp = flags()
                        nc.tensor.matmul(
                            ps[:, 0:R, 0:61],
                            wb[0:K, p, 128:256],
                            xt[0:K, r0 + dlt : r0 + dlt + R, 2:63],
                            start=st, stop=sp,
                        )
                    # kw4: M=64 col-tiled halves, A->half0 aligned, B->half1 +1col
                    for p in range(KH):
                        st, sp = flags()
                        nc.tensor.matmul(
                            ps[0:64, 0:R, 0:60],
                            wa[:, p, 256:320],
                            xa[:, r0 + p : r0 + p + R, 4:64],
                            start=st, stop=sp,
                        )
                        if p < 4:
                            t, dlt, K = B_PASSES[p]
                            xt = xbs[t]
                            st, sp = flags()
                            nc.tensor.matmul(
                                ps[64:128, 0:R, 1:61],
                                wb[0:K, p, 256:320],
                                xt[0:K, r0 + dlt : r0 + dlt + R, 4:64],
                                start=st, stop=sp,
                            )
                    assert k == n_mm

                    t_sb = tpool.tile([O, RCH, 60], dt.float32, tag="t")
                    ot = opool.tile([O, RCH, 60], dt.float32, tag="out")
                    nc.scalar.add(
                        t_sb[:, 0:R, :], ps[0:64, 0:R, 0:60], bias_sb[:, 0:1]
                    )
                    nc.vector.tensor_add(
                        ot[:, 0:R, :], t_sb[:, 0:R, :], ps[64:128, 0:R, 1:61]
                    )
                    nc.sync.dma_start(out_d[:, i, r0 : r0 + R, :], ot[:, 0:R, :])
    nc.compile()
    return nc


def _prep_inputs(x, weight, bias, idx):
    x16 = np.asarray(x).astype(BF16)  # [B, C, G_IN, X, Y]
    w = np.asarray(weight).astype(np.float32)  # [O, C, G_F, KH, KW]

    # wa[kh, lane*32+c, m]: lanes f0..f3; m = [kw0|kw1 (128), kw2|kw3 (128), kw4 (64)]
    wa = np.zeros((KH, 128, 320), np.float32)
    for kh in range(KH):
        for f in range(4):
            for kw in range(KW):
                blk = w[:, :, f, kh, kw].T  # [C, O]
                if kw < 4:
                    wa[kh, f * 32 : f * 32 + 32, kw * 64 : kw * 64 + 64] = blk
                else:
                    wa[kh, f * 32 : f * 32 + 32, 256:320] = blk

    # wb[p, lane*32+c, m]: per B_PASSES; lane tap kh = delta + shift
    wb = np.zeros((4, 128, 320), np.float32)
    for p, (t, dlt, K) in enumerate(B_PASSES):
        lanes = B_TILE_LANES[t]
        for l, (f, s) in enumerate(lanes):
            if l * 32 >= K:
                continue
            kh = dlt + s
            if kh >= KH:
                continue
            for kw in range(KW):
                blk = w[:, :, f, kh, kw].T
                if kw < 4:
                    wb[p, l * 32 : l * 32 + 32, kw * 64 : kw * 64 + 64] = blk
                else:
                    wb[p, l * 32 : l * 32 + 32, 256:320] = blk

    wa16 = np.ascontiguousarray(wa.transpose(1, 0, 2)).astype(BF16)
    wb16 = np.ascontiguousarray(wb.transpose(1, 0, 2)).astype(BF16)
    b2 = np.ascontiguousarray(np.asarray(bias).astype(np.float32).reshape(O, 1))
    in_maps = []
    for b in range(B):
        in_maps.append(
            {
                "x": np.ascontiguousarray(x16[b]),
                "wa": wa16,
                "wb": wb16,
                "bias": b2,
            }
        )
    return in_maps


def run(x, weight, bias, idx, trace=False):
    idx = np.asarray(idx).astype(np.int64)
    assert idx.shape == (G_OUT, G_F) and idx.min() >= 0 and idx.max() < G_IN
    nc = _build_nc(idx)
    in_maps = _prep_inputs(x, weight, bias, idx)
    res = run_bass_kernel_spmd(nc, in_maps, list(range(B)), trace=trace)
    out = np.stack([res.results[b]["out"] for b in range(B)]).astype(np.float32)
    return out, res


def kernel(x, weight, bias, idx):
    out, _ = run(x, weight, bias, idx, trace=False)
    return out
